# revision 10
# baseline (speedup 1.0000x reference)
"""Linear attention (non-causal, elu+1 feature map) on 8 Trainium2 cores — v6.

Math per (batch b, head h), phi(x) = elu(x)+1:
    C_aug = phi(K)^T @ [V | 1]        # (64, 65): context + k_sum col
    numer = phi(Q) @ C_aug[:, :64]
    denom = phi(Q) @ C_aug[:, 64]
    out   = numer / denom             # eps=1e-6 negligible vs denom ~1e5

Key choices vs the fp32 baseline (233us):
  * fp16 inputs (host casts): PE matmuls at 1 cycle/row instead of 4, one
    LDWEIGHTS pass instead of two, half the HBM traffic (33MB -> 16.3MB per
    core; DMA roofline ~46-50us).
  * Both heads fused per matmul. Host packs [K0|K1|V0|1|V1|1] (258 cols per
    t-row) so mm1's stationary (128 K-cols) and moving (130 V-cols) APs are
    single-stride; psum diag blocks give C0_aug/C1_aug. mm2 streams a
    block-diagonal 128x130 C against contiguous 128-col phiQ chunks.
  * phi is never materialized: phi(x) = exp(min(x,0)) + relu(x), and the PE
    adds the two halves by accumulating two matmuls into the same psum
    (scalar_tensor_tensor has no DVE fast mode — 1.08ns/elem — while
    min/max tensor_scalar runs at 0.31ns/elem and the extra matmul pass is
    ~70ns; measured on-HW).
  * mm2 lhsT chunks are contiguous (t = 128j + p); output is one interleaved
    (p, j, h, e) bf16 tile per batch, host un-permutes. bf16 output never
    goes subnormal at our magnitudes (fp16 would below 6e-5).
  * Engine split (TRN2 Pool does no tensor arithmetic): DVE does min/relu
    (4x fp16 mode) + reciprocal + the normalize multiply (one stride-0
    broadcast scalar_tensor_tensor per 3-chunk psum group); Act does the two
    exp passes and the C diag-block casts; Pool only memsets.
  * Two-stage software pipeline in EMISSION order (engine sequencers run
    in-order): stage A(b) = loads + phi + mm1, stage B(b) = C-cast + mm2 +
    normalize + output, emitted A0 A1 B0 A2 B1 A3 B2 B3 so batch b's
    normalize (which waits on mm2 psum) never blocks batch b+1's phi on the
    DVE, and the PE always has mm1(b+1) queued behind mm2(b).
  * Input DMAs split in quarters, output DMA in quarters, to shorten
    pipeline fill/drain and spread queue load.

Accuracy: fp16 quantization of phi(K),V gives C entries ~0.2% rms error;
through the normalizer this lands ~1.4e-4 absolute worst-case on outputs vs
the 2e-2 per-element gate with its 1e-3 floor (measured 1.52e-2 max rel).
"""

from contextlib import ExitStack

import numpy as np

import concourse.bacc as bacc
import concourse.bass as bass
import concourse.mybir as mybir
import concourse.tile as tile
from concourse.bass_utils import run_bass_kernel_spmd

B = 4
T = 4096
D = 1024
H = 16
E = 64
EA = E + 1
NCORES = 8
HPC = H // NCORES  # 2 heads per core
KC = HPC * E  # 128 packed K columns per t-row
W2 = KC + HPC * EA  # 258 cols per kva row: [K0|K1|V0|1|V1|1]
P = 128
NT = T // P  # 32 t-tiles for mm1 (t = p*32 + n)
NJ = T // P  # 32 t-chunks for mm2 (t = 128*j + p)
F16 = mybir.dt.float16
F32 = mybir.dt.float32
BF16 = mybir.dt.bfloat16
AF = mybir.ActivationFunctionType
ALU = mybir.AluOpType

# mm2 psum grouping: chunks per tile (3*130*4B = 1560B <= 2KB bank).
# Grouped so each run of 8 chunks (one output-DMA quarter) closes cleanly.
GRPS = [3, 3, 2, 3, 3, 2, 3, 3, 2, 3, 3, 2]
assert sum(GRPS) == NJ


def build_nc():
    nc = bacc.Bacc("TRN2", target_bir_lowering=False, debug=False)
    qt = nc.dram_tensor("qt", [B, P, T], F16, kind="ExternalInput").ap()
    kva = nc.dram_tensor("kva", [B, T, W2], F16, kind="ExternalInput").ap()
    o = nc.dram_tensor("o", [B, P, NJ * HPC * E], BF16, kind="ExternalOutput").ap()

    with tile.TileContext(nc) as tc, ExitStack() as ctx:
        qt_pool = ctx.enter_context(tc.tile_pool(name="qt", bufs=3))
        kv_pool = ctx.enter_context(tc.tile_pool(name="kv", bufs=2))
        eq_pool = ctx.enter_context(tc.tile_pool(name="eq", bufs=6))
        tk_pool = ctx.enter_context(tc.tile_pool(name="tk", bufs=4))
        c_pool = ctx.enter_context(tc.tile_pool(name="c", bufs=2))
        out_pool = ctx.enter_context(tc.tile_pool(name="out", bufs=2))
        r_pool = ctx.enter_context(tc.tile_pool(name="r", bufs=8))
        psc_pool = ctx.enter_context(tc.tile_pool(name="psc", bufs=2, space="PSUM"))
        pso_pool = ctx.enter_context(tc.tile_pool(name="pso", bufs=6, space="PSUM"))

        HW = NT * W2  # 8256 elems per partition
        TQ = T // 4  # 1024 cols per quarter
        NQ = NT // 4  # 8 n-tiles per quarter

        state = {}

        def emit_stage_a(b):
            # ---- Q^T load; E_q = exp(min(q,0)) in tmp, R_q = relu(q) ----
            qt_t = qt_pool.tile([P, T], F16)
            eqs = []
            for q4 in range(4):
                sl = slice(q4 * TQ, (q4 + 1) * TQ)
                nc.sync.dma_start(qt_t[:, sl], qt[b, :, sl])
                x = qt_t[:, sl]
                tq = eq_pool.tile([P, TQ], F16)
                nc.vector.tensor_scalar_min(tq[:], x, 0.0)
                nc.scalar.activation(tq[:], tq[:], AF.Exp)
                nc.vector.tensor_scalar_max(x, x, 0.0)
                eqs.append(tq)

            kv = kv_pool.tile([P, HW], F16)
            kvr = kv[:].rearrange("p (n c) -> p n c", c=W2)
            psc = psc_pool.tile([P, HPC * EA], F32)
            tks = []
            for q4 in range(4):
                nsl = slice(q4 * NQ, (q4 + 1) * NQ)
                csl = slice(q4 * (HW // 4), (q4 + 1) * (HW // 4))
                nc.sync.dma_start(
                    kv[:, csl],
                    kva[b].rearrange("(p n) c -> p (n c)", p=P)[:, csl],
                )
                kview = kvr[:, nsl, 0:KC]
                tk = tk_pool.tile([P, NQ * KC], F16)
                tk3 = tk[:].rearrange("p (n c) -> p n c", c=KC)
                nc.vector.tensor_scalar_min(tk3, kview, 0.0)
                nc.scalar.activation(tk3, tk3, AF.Exp)
                nc.vector.tensor_scalar_max(kview, kview, 0.0)
                tks.append(tk)
                for n in range(q4 * NQ, (q4 + 1) * NQ):
                    vaug = kvr[:, n, KC:W2]
                    nc.tensor.matmul(
                        psc[:],
                        lhsT=tk[:, (n - q4 * NQ) * KC : (n - q4 * NQ + 1) * KC],
                        rhs=vaug,
                        start=(n == 0),
                        stop=False,
                    )
                    nc.tensor.matmul(
                        psc[:],
                        lhsT=kvr[:, n, 0:KC],
                        rhs=vaug,
                        start=False,
                        stop=(n == NT - 1),
                    )
            state[b] = (qt_t, eqs, psc)

        def emit_stage_b(b):
            qt_t, eqs, psc = state[b]
            # ---- block-diagonal C for the fused mm2 ----
            c_sb = c_pool.tile([P, HPC * EA], F16)
            nc.gpsimd.memset(c_sb[:], 0.0)
            nc.scalar.copy(c_sb[0:E, 0:EA], psc[0:E, 0:EA])
            nc.scalar.copy(c_sb[E:P, EA : 2 * EA], psc[E:P, EA : 2 * EA])

            # ---- mm2 (E and R accumulated) + normalize + streamed output ----
            ob = out_pool.tile([P, NJ * HPC * E], BF16)
            j = 0
            for gi, grp in enumerate(GRPS):
                ps = pso_pool.tile([P, grp * HPC * EA], F32)
                for k in range(grp):
                    jj = j + k
                    q4, jq = jj // 8, jj % 8
                    blk = ps[:, k * HPC * EA : (k + 1) * HPC * EA]
                    nc.tensor.matmul(
                        blk,
                        lhsT=eqs[q4][:, jq * P : (jq + 1) * P],
                        rhs=c_sb[:],
                        start=True,
                        stop=False,
                    )
                    nc.tensor.matmul(
                        blk,
                        lhsT=qt_t[:, jj * P : (jj + 1) * P],
                        rhs=c_sb[:],
                        start=False,
                        stop=True,
                    )
                r = r_pool.tile([P, grp * HPC], F32)
                nc.vector.reciprocal(r[:], ps[:, E::EA])
                numer = ps[:].rearrange("p (k h c) -> p k h c", k=grp, h=HPC)[
                    :, :, :, 0:E
                ]
                rb = r[:].rearrange("p (k h c) -> p k h c", k=grp, h=HPC)
                numer_b, rb = bass.broadcast_tensor_aps(numer, rb)
                oview = ob[
                    :, j * HPC * E : (j + grp) * HPC * E
                ].rearrange("p (k h c) -> p k h c", k=grp, h=HPC)
                nc.vector.scalar_tensor_tensor(
                    oview, numer_b, 1.0, rb, ALU.mult, ALU.mult
                )
                j += grp
                if gi % 3 == 2:  # a quarter (8 chunks) of ob is complete
                    q4 = gi // 3
                    sl = slice(q4 * 8 * HPC * E, (q4 + 1) * 8 * HPC * E)
                    nc.sync.dma_start(o[b][:, sl], ob[:, sl])

        emit_stage_a(0)
        emit_stage_a(1)
        emit_stage_b(0)
        emit_stage_a(2)
        emit_stage_b(1)
        emit_stage_a(3)
        emit_stage_b(2)
        emit_stage_b(3)
    nc.finalize()
    return nc


_NC_CACHE = None


def _get_nc():
    global _NC_CACHE
    if _NC_CACHE is None:
        _NC_CACHE = build_nc()
    return _NC_CACHE


def make_in_maps(query, key, value):
    query = np.asarray(query, dtype=np.float32)
    key = np.asarray(key, dtype=np.float32)
    value = np.asarray(value, dtype=np.float32)
    in_maps = []
    for c in range(NCORES):
        lo = c * HPC * E
        hi = lo + HPC * E
        qt = np.ascontiguousarray(
            query[:, :, lo:hi].transpose(0, 2, 1), dtype=np.float16
        )
        kva = np.empty((B, T, W2), np.float16)
        kva[..., 0:KC] = key[:, :, lo:hi]
        kva[..., KC : KC + E] = value[:, :, lo : lo + E]
        kva[..., KC + E] = 1.0
        kva[..., KC + EA : KC + EA + E] = value[:, :, lo + E : hi]
        kva[..., KC + EA + E] = 1.0
        in_maps.append({"qt": qt, "kva": kva})
    return in_maps


def assemble_out(results):
    out = np.empty((B, T, D), np.float32)
    for c in range(NCORES):
        # o[b, p, ((j*2 + h)*64 + e)] = out[b, t=128j+p, c*128 + h*64 + e]
        oc = np.asarray(results[c]["o"], dtype=np.float32)
        oc = oc.reshape(B, P, NJ, HPC, E).transpose(0, 2, 1, 3, 4)
        out[:, :, c * HPC * E : (c + 1) * HPC * E] = oc.reshape(B, T, HPC * E)
    return out


def run(query, key, value, **spmd_kwargs):
    nc = _get_nc()
    in_maps = make_in_maps(query, key, value)
    res = run_bass_kernel_spmd(nc, in_maps, core_ids=list(range(NCORES)), **spmd_kwargs)
    return assemble_out(res.results), res


def kernel(query, key, value):
    out, _ = run(query, key, value)
    return out


# revision 11
# speedup vs baseline: 1.0447x; 1.0447x over previous
"""Linear attention (non-causal, elu+1 feature map) on 8 Trainium2 cores — v7.

Math per (batch b, head h), phi(x) = elu(x)+1:
    C_aug = phi(K)^T @ [V | 1]        # (64, 65): context + k_sum col
    numer = phi(Q) @ C_aug[:, :64]
    denom = phi(Q) @ C_aug[:, 64]
    out   = numer / denom             # eps=1e-6 negligible vs denom ~1e5

Key choices vs the fp32 baseline (233us):
  * fp16 inputs (host casts): PE matmuls at 1 cycle/row instead of 4, one
    LDWEIGHTS pass instead of two, half the HBM traffic (33MB -> 16.3MB per
    core; DMA roofline ~46-50us).
  * Both heads fused per matmul. Host packs [K0|K1|V0|1|V1|1] (258 cols per
    t-row) so mm1's stationary (128 K-cols) and moving (130 V-cols) APs are
    single-stride; psum diag blocks give C0_aug/C1_aug. mm2 streams a
    block-diagonal 128x130 C against contiguous 128-col phiQ chunks.
  * phi is never materialized: phi(x) = exp(min(x,0)) + relu(x), and the PE
    adds the two halves by accumulating two matmuls into the same psum
    (scalar_tensor_tensor has no DVE fast mode — 1.08ns/elem — while
    min/max tensor_scalar runs at 0.31ns/elem; measured on-HW).
  * mm2 lhsT chunks are contiguous (t = 128j + p); output is one interleaved
    (p, j, h, e) bf16 tile per batch, host un-permutes. bf16 output never
    goes subnormal at our magnitudes (fp16 would below 6e-5).
  * Engine split: DVE does minQ/reluQ/minK (4x fp16 mode) + reciprocal + the
    normalize multiply (stride-0 broadcast scalar_tensor_tensor per psum
    group); Act does both exp passes, relu(K), and the C diag-block casts.
  * Output DMAs are triggered from the (otherwise idle) Pool DGE queue —
    on the SP queue they would head-of-line block the next batches' input
    prefetch behind the normalize dependency.
  * Three-stage software pipeline in EMISSION order (engine sequencers are
    in-order): L(b)=loads+phi, M(b)=mm1, B(b)=C-cast+mm2+normalize+output,
    emitted L0 M0 L1 B0 M1 L2 B1 M2 L3 B2 M3 B3. PE order stays
    mm1(b),mm2(b),mm1(b+1); DVE does phi(b+1) before norm(b); Act does
    exp(b+1) before the C-cast of b.

Accuracy: fp16 quantization of phi(K),V gives C entries ~0.2% rms error;
through the normalizer this lands ~1.4e-4 absolute worst-case on outputs vs
the 2e-2 per-element gate with its 1e-3 floor (measured 1.52e-2 max rel).
"""

from contextlib import ExitStack

import numpy as np

import concourse.bacc as bacc
import concourse.bass as bass
import concourse.mybir as mybir
import concourse.tile as tile
from concourse.bass_utils import run_bass_kernel_spmd

B = 4
T = 4096
D = 1024
H = 16
E = 64
EA = E + 1
NCORES = 8
HPC = H // NCORES  # 2 heads per core
KC = HPC * E  # 128 packed K columns per t-row
W2 = KC + HPC * EA  # 258 cols per kva row: [K0|K1|V0|1|V1|1]
P = 128
NT = T // P  # 32 t-tiles for mm1 (t = p*32 + n)
NJ = T // P  # 32 t-chunks for mm2 (t = 128*j + p)
F16 = mybir.dt.float16
F32 = mybir.dt.float32
BF16 = mybir.dt.bfloat16
AF = mybir.ActivationFunctionType
ALU = mybir.AluOpType

# mm2 psum grouping: chunks per tile (3*130*4B = 1560B <= 2KB bank).
# Grouped so each run of 8 chunks (one output-DMA quarter) closes cleanly.
GRPS = [3, 3, 2, 3, 3, 2, 3, 3, 2, 3, 3, 2]
assert sum(GRPS) == NJ


def build_nc():
    nc = bacc.Bacc("TRN2", target_bir_lowering=False, debug=False)
    qt = nc.dram_tensor("qt", [B, P, T], F16, kind="ExternalInput").ap()
    kva = nc.dram_tensor("kva", [B, T, W2], F16, kind="ExternalInput").ap()
    o = nc.dram_tensor("o", [B, P, NJ * HPC * E], BF16, kind="ExternalOutput").ap()

    with tile.TileContext(nc) as tc, ExitStack() as ctx:
        qt_pool = ctx.enter_context(tc.tile_pool(name="qt", bufs=3))
        kv_pool = ctx.enter_context(tc.tile_pool(name="kv", bufs=2))
        eq_pool = ctx.enter_context(tc.tile_pool(name="eq", bufs=12))
        tk_pool = ctx.enter_context(tc.tile_pool(name="tk", bufs=8))
        c_pool = ctx.enter_context(tc.tile_pool(name="c", bufs=2))
        out_pool = ctx.enter_context(tc.tile_pool(name="out", bufs=2))
        r_pool = ctx.enter_context(tc.tile_pool(name="r", bufs=8))
        psc_pool = ctx.enter_context(tc.tile_pool(name="psc", bufs=2, space="PSUM"))
        pso_pool = ctx.enter_context(tc.tile_pool(name="pso", bufs=6, space="PSUM"))

        HW = NT * W2  # 8256 elems per partition
        TQ = T // 4  # 1024 cols per quarter
        NQ = NT // 4  # 8 n-tiles per quarter

        state = {}

        def emit_load_phi(b):
            # Q^T load; E_q = exp(min(q,0)) into eq tiles, R_q = relu(q)
            qt_t = qt_pool.tile([P, T], F16)
            eqs = []
            for q4 in range(4):
                sl = slice(q4 * TQ, (q4 + 1) * TQ)
                nc.sync.dma_start(qt_t[:, sl], qt[b, :, sl])
                x = qt_t[:, sl]
                tq = eq_pool.tile([P, TQ], F16)
                nc.vector.tensor_scalar_min(tq[:], x, 0.0)
                nc.scalar.activation(tq[:], tq[:], AF.Exp)
                nc.vector.tensor_scalar_max(x, x, 0.0)
                eqs.append(tq)

            # [K0|K1|V0|1|V1|1] load; E_k into tk tiles, R_k in place
            kv = kv_pool.tile([P, HW], F16)
            kvr = kv[:].rearrange("p (n c) -> p n c", c=W2)
            tks = []
            for q4 in range(4):
                nsl = slice(q4 * NQ, (q4 + 1) * NQ)
                csl = slice(q4 * (HW // 4), (q4 + 1) * (HW // 4))
                nc.sync.dma_start(
                    kv[:, csl],
                    kva[b].rearrange("(p n) c -> p (n c)", p=P)[:, csl],
                )
                kview = kvr[:, nsl, 0:KC]
                tk = tk_pool.tile([P, NQ * KC], F16)
                tk3 = tk[:].rearrange("p (n c) -> p n c", c=KC)
                nc.vector.tensor_scalar_min(tk3, kview, 0.0)
                nc.scalar.activation(tk3, tk3, AF.Exp)
                nc.scalar.activation(kview, kview, AF.Relu)
                tks.append(tk)
            state[b] = (qt_t, eqs, kv, kvr, tks)

        def emit_mm1(b):
            qt_t, eqs, kv, kvr, tks = state[b]
            psc = psc_pool.tile([P, HPC * EA], F32)
            for n in range(NT):
                q4, nq = n // NQ, n % NQ
                vaug = kvr[:, n, KC:W2]
                nc.tensor.matmul(
                    psc[:],
                    lhsT=tks[q4][:, nq * KC : (nq + 1) * KC],
                    rhs=vaug,
                    start=(n == 0),
                    stop=False,
                )
                nc.tensor.matmul(
                    psc[:],
                    lhsT=kvr[:, n, 0:KC],
                    rhs=vaug,
                    start=False,
                    stop=(n == NT - 1),
                )
            state[b] = (qt_t, eqs, psc)

        def emit_tail(b):
            qt_t, eqs, psc = state[b]
            # block-diagonal C for the fused mm2
            c_sb = c_pool.tile([P, HPC * EA], F16)
            nc.gpsimd.memset(c_sb[:], 0.0)
            nc.scalar.copy(c_sb[0:E, 0:EA], psc[0:E, 0:EA])
            nc.scalar.copy(c_sb[E:P, EA : 2 * EA], psc[E:P, EA : 2 * EA])

            # mm2 (E and R accumulated) + normalize + streamed output
            ob = out_pool.tile([P, NJ * HPC * E], BF16)
            j = 0
            for gi, grp in enumerate(GRPS):
                ps = pso_pool.tile([P, grp * HPC * EA], F32)
                for k in range(grp):
                    jj = j + k
                    q4, jq = jj // 8, jj % 8
                    blk = ps[:, k * HPC * EA : (k + 1) * HPC * EA]
                    nc.tensor.matmul(
                        blk,
                        lhsT=eqs[q4][:, jq * P : (jq + 1) * P],
                        rhs=c_sb[:],
                        start=True,
                        stop=False,
                    )
                    nc.tensor.matmul(
                        blk,
                        lhsT=qt_t[:, jj * P : (jj + 1) * P],
                        rhs=c_sb[:],
                        start=False,
                        stop=True,
                    )
                r = r_pool.tile([P, grp * HPC], F32)
                nc.vector.reciprocal(r[:], ps[:, E::EA])
                numer = ps[:].rearrange("p (k h c) -> p k h c", k=grp, h=HPC)[
                    :, :, :, 0:E
                ]
                rb = r[:].rearrange("p (k h c) -> p k h c", k=grp, h=HPC)
                numer_b, rb = bass.broadcast_tensor_aps(numer, rb)
                oview = ob[
                    :, j * HPC * E : (j + grp) * HPC * E
                ].rearrange("p (k h c) -> p k h c", k=grp, h=HPC)
                nc.vector.scalar_tensor_tensor(
                    oview, numer_b, 1.0, rb, ALU.mult, ALU.mult
                )
                j += grp
                if gi % 3 == 2:  # a quarter (8 chunks) of ob is complete
                    q4 = gi // 3
                    sl = slice(q4 * 8 * HPC * E, (q4 + 1) * 8 * HPC * E)
                    nc.gpsimd.dma_start(o[b][:, sl], ob[:, sl])

        emit_load_phi(0)
        emit_mm1(0)
        emit_load_phi(1)
        emit_tail(0)
        emit_mm1(1)
        emit_load_phi(2)
        emit_tail(1)
        emit_mm1(2)
        emit_load_phi(3)
        emit_tail(2)
        emit_mm1(3)
        emit_tail(3)
    nc.finalize()
    return nc


_NC_CACHE = None


def _get_nc():
    global _NC_CACHE
    if _NC_CACHE is None:
        _NC_CACHE = build_nc()
    return _NC_CACHE


def make_in_maps(query, key, value):
    query = np.asarray(query, dtype=np.float32)
    key = np.asarray(key, dtype=np.float32)
    value = np.asarray(value, dtype=np.float32)
    in_maps = []
    for c in range(NCORES):
        lo = c * HPC * E
        hi = lo + HPC * E
        qt = np.ascontiguousarray(
            query[:, :, lo:hi].transpose(0, 2, 1), dtype=np.float16
        )
        kva = np.empty((B, T, W2), np.float16)
        kva[..., 0:KC] = key[:, :, lo:hi]
        kva[..., KC : KC + E] = value[:, :, lo : lo + E]
        kva[..., KC + E] = 1.0
        kva[..., KC + EA : KC + EA + E] = value[:, :, lo + E : hi]
        kva[..., KC + EA + E] = 1.0
        in_maps.append({"qt": qt, "kva": kva})
    return in_maps


def assemble_out(results):
    out = np.empty((B, T, D), np.float32)
    for c in range(NCORES):
        # o[b, p, ((j*2 + h)*64 + e)] = out[b, t=128j+p, c*128 + h*64 + e]
        oc = np.asarray(results[c]["o"], dtype=np.float32)
        oc = oc.reshape(B, P, NJ, HPC, E).transpose(0, 2, 1, 3, 4)
        out[:, :, c * HPC * E : (c + 1) * HPC * E] = oc.reshape(B, T, HPC * E)
    return out


def run(query, key, value, **spmd_kwargs):
    nc = _get_nc()
    in_maps = make_in_maps(query, key, value)
    res = run_bass_kernel_spmd(nc, in_maps, core_ids=list(range(NCORES)), **spmd_kwargs)
    return assemble_out(res.results), res


def kernel(query, key, value):
    out, _ = run(query, key, value)
    return out
